# revision 66
# baseline (speedup 1.0000x reference)
"""CQAttention Trainium2 kernel (v4).

Reference per batch b (C:[D,Lc], Q:[D,Lq], D=128, Lc=2048, Lq=512):
    Ct = C^T, Qt = Q^T
    S  = Ct@w4C + (Qt@w4Q)^T + (Ct*w4mlu)@Qt^T + bias        [Lc, Lq]
    S1 = softmax_q(S + NEG*(1-qmask)),  S2 = softmax_c(S + NEG*(1-cmask))
    A  = S1 @ Qt;  B = S1 @ (S2^T @ Ct)
    out= transpose(concat([Ct, A, Ct*A, Ct*B], -1))           [4D, Lc]

Kernel algebra (single exp pass; one full-S exponential family):
    qterm[q] = q1[q] + qneg[q]   (rank-0/1 matmuls; PSUM-seeded per S tile)
    X[c,q]   = exp(sub2[c,q] + qterm[q] + c0[c] + cneg[c])
    rowsum[c]= sum_q X[c,q]              (free via ActE accum_out on the exp)
    S1t[q,c] = X^T[q,c] / rowsum[c]      (diag(rcp) matmul-transpose: a plain
                                          bf16 matmul, stationary X-block
                                          shared with the R matmuls)
    R        = (X wts vs [Ct|1]) -> rp/(s2+eps)
    A^T      = Qt^T @ S1t ;  B^T = R^T @ S1t
  `bias` cancels in both softmaxes (dropped exactly). The per-c factor
  exp(c0+cneg) cancels in S1's rowsum division; per-q factor exp(qterm)
  cancels in R's s2 division, so one exp family serves both softmaxes.
  qmask semantics are exact; for cmask=0 columns A/CA/CB deviate from the
  reference (0 instead of unmasked-softmax values); the input spec fixes
  Cmask=ones.

Scheduling: the program is emitted in two passes (loads/prep/S for all
batches, then R/transpose/AB for all batches) so that per-name tile-pool
slot rotation lets batch 1's front overlap batch 0's tail. CA/CB products
run on the otherwise-idle Pool engine from SBUF copies of A/B (the A copy
also feeds the output DMA).
"""

import numpy as np
from contextlib import ExitStack

import concourse.bass as bass
import concourse.mybir as mybir
import concourse.tile as tile
from concourse import bacc
from concourse.bass_utils import run_bass_kernel_spmd
from concourse.masks import make_identity

F32 = mybir.dt.float32
F32R = mybir.dt.float32r
I32 = mybir.dt.int32
BF16 = mybir.dt.bfloat16
AF = mybir.ActivationFunctionType
ALU = mybir.AluOpType

B, D, LC, LQ = 16, 128, 2048, 512
NCORES = 8
BL = B // NCORES          # batches per core
NEG = -1e30
NCT = LC // 128           # 16 c-tiles
NQT = LQ // 128           # 4 q-tiles
NCJ = LC // 512           # 4 c-chunks (free-dim)
CTS = 130                 # Ct slot: [Ct(128) | ones(1) | pad(1)]
EPS = 1e-30


def _build_nc():
    nc = bacc.Bacc("TRN2", target_bir_lowering=False)
    Ci = nc.dram_tensor("C", [BL, D, LC], F32, kind="ExternalInput")
    Qi = nc.dram_tensor("Q", [BL, D, LQ], F32, kind="ExternalInput")
    CMi = nc.dram_tensor("Cmask", [BL, LC], I32, kind="ExternalInput")
    QMi = nc.dram_tensor("Qmask", [BL, LQ], I32, kind="ExternalInput")
    w4C = nc.dram_tensor("w4C", [D, 1], F32, kind="ExternalInput")
    w4Q = nc.dram_tensor("w4Q", [D, 1], F32, kind="ExternalInput")
    w4mlu = nc.dram_tensor("w4mlu", [1, 1, D], F32, kind="ExternalInput")
    biasi = nc.dram_tensor("bias", [1], F32, kind="ExternalInput")
    out = nc.dram_tensor("out", [BL, 4 * D, LC], F32, kind="ExternalOutput")

    with tile.TileContext(nc) as tc, ExitStack() as ctx:
        const = ctx.enter_context(tc.tile_pool(name="const", bufs=1))
        sb2 = ctx.enter_context(tc.tile_pool(name="sb2", bufs=2))
        sb3 = ctx.enter_context(tc.tile_pool(name="sb3", bufs=3))
        # PSUM budget (8 banks): sp2 + pt2 + pab2 + sm1 + rp1
        ps_s = ctx.enter_context(tc.tile_pool(name="ps_s", bufs=2, space="PSUM"))
        ps_t = ctx.enter_context(tc.tile_pool(name="ps_t", bufs=2, space="PSUM"))
        ps_ab = ctx.enter_context(tc.tile_pool(name="ps_ab", bufs=2, space="PSUM"))
        ps_sm = ctx.enter_context(tc.tile_pool(name="ps_sm", bufs=1, space="PSUM"))
        ps_r = ctx.enter_context(tc.tile_pool(name="ps_r", bufs=1, space="PSUM"))

        # ---- constants (shared across batches) ----
        wmlu_sb = const.tile([D, 1], F32, name="wmlu_sb")
        nc.gpsimd.dma_start(out=wmlu_sb, in_=w4mlu.ap().rearrange("a b d -> d (a b)"))
        w4C_sb = const.tile([D, 1], F32R, name="w4C_sb")
        nc.gpsimd.dma_start(out=w4C_sb, in_=w4C[:, :].bitcast(F32R))
        w4Q_sb = const.tile([D, 1], F32R, name="w4Q_sb")
        nc.scalar.dma_start(out=w4Q_sb, in_=w4Q[:, :].bitcast(F32R))
        ident0 = const.tile([D, D], F32, name="ident0")
        make_identity(nc, ident0)
        identR = const.tile([D, D], F32R, name="identR")
        nc.vector.tensor_copy(identR, ident0)
        identB = const.tile([D, D], BF16, name="identB")
        nc.vector.tensor_copy(identB, ident0)
        id16 = const.tile([16, 16], F32, name="id16")
        make_identity(nc, id16)
        ones_f = const.tile([1, D], F32, name="ones_f")
        nc.vector.memset(ones_f, 1.0)
        one1R = const.tile([1, 1], F32R, name="one1R")
        nc.vector.tensor_copy(one1R, ones_f[:, 0:1])
        ones_row = const.tile([1, D], F32R, name="ones_row")
        nc.vector.tensor_copy(ones_row, ones_f)
        ones_col = const.tile([D, 1], F32, name="ones_col")
        nc.vector.memset(ones_col, 1.0)

        st = [dict() for _ in range(BL)]

        def emit_loads(b):
            s = st[b]
            # All loads ride the sync queue (SP has no compute to block),
            # masks first: they head the bias/qterm dependency chains.
            C_sb = s["C_sb"] = sb2.tile([D, LC], F32R, name="C_sb")
            Q_sb = s["Q_sb"] = sb2.tile([D, LQ], F32R, name="Q_sb")
            nc.sync.dma_start(out=Q_sb, in_=Qi[b, :, :].bitcast(F32R))
            if b == 0:
                nc.sync.dma_start(out=C_sb[:, 0:512],
                                  in_=Ci[b, :, 0:512].bitcast(F32R))
            s["qm_row"] = sb2.tile([1, LQ], I32, name="qm_row")
            nc.sync.dma_start(out=s["qm_row"], in_=QMi[b, :].unsqueeze(0))
            s["cm_pi"] = sb2.tile([16, 128], I32, name="cm_pi")
            nc.gpsimd.dma_start(out=s["cm_pi"],
                                in_=CMi[b, :].rearrange("(p i) -> p i", p=16))
            if b == 0:
                nc.sync.dma_start(out=C_sb[:, 512:2048],
                                  in_=Ci[b, :, 512:2048].bitcast(F32R))
            else:
                nc.sync.dma_start(out=C_sb, in_=Ci[b, :, :].bitcast(F32R))

        def emit_prep(b):
            # Q/mask-dependent prep (ready before C lands)
            s = st[b]
            Q_sb = s["Q_sb"]
            qneg_row = sb2.tile([1, LQ], F32R, name="qneg_row")
            nc.vector.tensor_scalar(qneg_row, s["qm_row"], -NEG, NEG,
                                    op0=ALU.mult, op1=ALU.add)
            cn_t = sb2.tile([16, 128], F32, name="cn_t")
            nc.vector.tensor_scalar(cn_t, s["cm_pi"], -NEG, NEG,
                                    op0=ALU.mult, op1=ALU.add)
            cnT_sb = s["cnT_sb"] = sb2.tile([128, 16], F32, name="cnT_sb")
            cnT_p = ps_sm.tile([128, 16], F32, name="sm")
            nc.tensor.transpose(cnT_p, cn_t, id16)
            nc.vector.tensor_copy(cnT_sb, cnT_p)
            qt_p = ps_sm.tile([1, LQ], F32, name="sm")
            nc.tensor.matmul(qt_p, w4Q_sb, Q_sb, start=True, stop=False)
            nc.tensor.matmul(qt_p, one1R, qneg_row, start=False, stop=True)
            qterm_row = s["qterm_row"] = sb2.tile([1, LQ], F32R,
                                                  name="qterm_row")
            nc.scalar.activation(qterm_row, qt_p, AF.Copy)

        def emit_sloop(b, work, chunk_hook=None):
            # S phase, chunk-interleaved with C's arrival; after each S tile
            # pop one deferred-work closure to fill PE's exp-cadence slack.
            s = st[b]
            C_sb, Q_sb, qterm_row = s["C_sb"], s["Q_sb"], s["qterm_row"]
            cnT_sb = s["cnT_sb"]
            bias_c = sb2.tile([128, NCT], F32, name="bias_c")
            Cw = sb2.tile([D, LC], F32R, name="Cw")
            X = s["X"] = sb2.tile([128, NCT, LQ], BF16, name="X")
            rowsum = sb2.tile([128, NCT], F32, name="rowsum")
            rcp = sb2.tile([128, NCT], F32, name="rcp")
            diags = s["diags"] = sb2.tile([128, NCT, 128], BF16, name="diags")
            for cj in range(NCJ):
                sl = slice(cj * 512, (cj + 1) * 512)
                qs = slice(cj * 4, (cj + 1) * 4)
                c0_p = ps_sm.tile([128, 4], F32, name="sm")
                for k in range(4):
                    ci = cj * 4 + k
                    nc.tensor.matmul(
                        c0_p[:, k : k + 1],
                        C_sb.bitcast(F32)[:, ci * 128 : (ci + 1) * 128],
                        w4C_sb.bitcast(F32), start=True, stop=True)
                nc.vector.tensor_tensor(bias_c[:, qs], c0_p, cnT_sb[:, qs],
                                        ALU.add)
                nc.vector.tensor_scalar_mul(Cw[:, sl],
                                            C_sb[:, sl].bitcast(F32),
                                            wmlu_sb[:, 0:1])
                for k in range(4):
                    ci = cj * 4 + k
                    sp = ps_s.tile([128, LQ], F32, name="sp")
                    nc.tensor.matmul(sp, ones_row, qterm_row,
                                     start=True, stop=False)
                    nc.tensor.matmul(sp, Cw[:, ci * 128 : (ci + 1) * 128],
                                     Q_sb, start=False, stop=True)
                    if b == 0:
                        # b0's spine has DVE slack: rowsum via tensor_reduce
                        # keeps the serial ActE exp chain 187ns/tile shorter
                        nc.scalar.activation(X[:, ci, :], sp, AF.Exp,
                                             bias=bias_c[:, ci : ci + 1],
                                             scale=1.0)
                        nc.vector.tensor_reduce(rowsum[:, ci : ci + 1],
                                                X[:, ci, :],
                                                mybir.AxisListType.X, ALU.add)
                    else:
                        nc.scalar.activation(X[:, ci, :], sp, AF.Exp,
                                             bias=bias_c[:, ci : ci + 1],
                                             scale=1.0,
                                             accum_out=rowsum[:, ci : ci + 1])
                    if work:
                        work.pop(0)()
                    if work:
                        work.pop(0)()
                # eps guard: all-masked rows divide 0/eps -> 0, not NaN
                nc.vector.tensor_scalar_add(rowsum[:, qs], rowsum[:, qs], EPS)
                nc.vector.reciprocal(rcp[:, qs], rowsum[:, qs])
                for ck in range(cj * 4, cj * 4 + 4):
                    nc.vector.tensor_scalar_mul(diags[:, ck, :], identB,
                                                rcp[:, ck : ck + 1])
                if chunk_hook is not None:
                    work.extend(chunk_hook(cj))
            return work

        def emit_ctqt(b):
            # Ct/Qt tiles (bf16, batched PSUM->SBUF copies)
            s = st[b]
            C_sb, Q_sb = s["C_sb"], s["Q_sb"]
            Ct_sb = s["Ct_sb"] = sb2.tile([128, NCT, CTS], BF16, name="Ct_sb")
            nc.gpsimd.tensor_copy(
                Ct_sb[:, :, 128:129],
                ones_col[:, 0:1].unsqueeze(1).to_broadcast((128, NCT, 1)))

            def quad(cq):
                def go():
                    tp = ps_t.tile([128, 4, 128], F32R, name="pt")
                    for k in range(4):
                        ci = cq * 4 + k
                        nc.tensor.transpose(tp[:, k, :],
                                            C_sb[:, ci * 128 : (ci + 1) * 128],
                                            identR)
                    nc.vector.tensor_copy(
                        Ct_sb[:, cq * 4 : (cq + 1) * 4, 0:128],
                        tp.bitcast(F32))
                return go

            def qt():
                def go():
                    Qt_sb = s["Qt_sb"] = sb2.tile([128, NQT, 128], BF16,
                                                  name="Qt_sb")
                    tpq = ps_t.tile([128, NQT, 128], F32R, name="pt")
                    for qi in range(NQT):
                        nc.tensor.transpose(
                            tpq[:, qi, :],
                            Q_sb[:, qi * 128 : (qi + 1) * 128], identR)
                    nc.vector.tensor_copy(Qt_sb, tpq.bitcast(F32))
                return go
            return [quad(cq) for cq in range(4)] + [qt()]

        def pass2_work(b, part):
            # Deferred R/S1t-transpose/A/B slices (~0.5-1us of PE each)
            s = st[b]
            C_sb = s["C_sb"]
            if "R_sb" not in s:
                s["R_sb"] = sb2.tile([128, NQT, 128], BF16, name="R_sb")
                s["rs2"] = sb2.tile([128, NQT], F32, name="rs2")
                s["S1t"] = sb2.tile([128, NQT, LC], BF16, name="S1t")
            R_sb, rs2, S1t = s["R_sb"], s["rs2"], s["S1t"]

            rphold = s.setdefault("rphold", {})

            def r_slice(qi, half):
                def go():
                    if half == 0:
                        rphold[qi] = ps_r.tile([128, CTS], F32, name="rp")
                    rp = rphold[qi]
                    for ci in range(half * 8, half * 8 + 8):
                        nc.tensor.matmul(rp,
                                         s["X"][:, ci, qi * 128 : (qi + 1) * 128],
                                         s["Ct_sb"][:, ci, 0:CTS],
                                         start=(ci == 0), stop=(ci == NCT - 1))
                    if half == 1:
                        nc.vector.tensor_scalar_add(rs2[:, qi : qi + 1],
                                                    rp[:, 128:129], EPS)
                        nc.vector.reciprocal(rs2[:, qi : qi + 1],
                                             rs2[:, qi : qi + 1])
                        nc.vector.tensor_scalar_mul(R_sb[:, qi, :],
                                                    rp[:, 0:128],
                                                    rs2[:, qi : qi + 1])
                return go

            def t_slice(cj, qi):
                def go():
                    if b == 1 and (cj + qi) % 2 == 1:
                        tp = ps_s.tile([128, 512], F32, name="sp", tag="sp")
                    else:
                        tp = ps_t.tile([128, 512], F32, name="pt")
                    for k in range(4):
                        ci = cj * 4 + k
                        nc.tensor.matmul(
                            tp[:, k * 128 : (k + 1) * 128],
                            s["X"][:, ci, qi * 128 : (qi + 1) * 128],
                            s["diags"][:, ci, :], start=True, stop=True)
                    on_dve = ((qi * NCJ + cj) % 4 != 3 if b == 0
                              else (qi + cj) % 2 == 0)
                    if on_dve:
                        nc.vector.tensor_copy(
                            S1t[:, qi, cj * 512 : (cj + 1) * 512], tp)
                    else:
                        nc.scalar.activation(
                            S1t[:, qi, cj * 512 : (cj + 1) * 512], tp,
                            AF.Copy)
                return go

            def pa_slice(cj):
                def go():
                    sl = slice(cj * 512, (cj + 1) * 512)
                    pa = ps_ab.tile([128, 512], F32, name="pab")
                    for qi in range(NQT):
                        nc.tensor.matmul(pa, s["Qt_sb"][:, qi, :],
                                         S1t[:, qi, sl],
                                         start=(qi == 0), stop=(qi == NQT - 1))
                    cab = sb3.tile([128, 2, 512], F32, name="cab")
                    if b == 0:
                        nc.vector.tensor_copy(cab[:, 0, :], pa)
                        nc.gpsimd.tensor_tensor(cab[:, 1, :],
                                                C_sb[:, sl].bitcast(F32),
                                                cab[:, 0, :], ALU.mult)
                    else:
                        nc.scalar.activation(cab[:, 0, :], pa, AF.Copy)
                        nc.gpsimd.tensor_tensor(cab[:, 1, :],
                                                C_sb[:, sl].bitcast(F32),
                                                cab[:, 0, :], ALU.mult)
                    nc.sync.dma_start(
                        out=out[b, 128:384, sl].rearrange("(r p) c -> p r c",
                                                          p=128),
                        in_=cab)
                return go

            def pb_slice(cj, lo=None, cw=512):
                def go():
                    sl = (slice(cj * 512, (cj + 1) * 512) if lo is None
                          else slice(lo, lo + cw))
                    pb = ps_ab.tile([128, 512], F32, name="pab")[:, 0:cw]
                    for qi in range(NQT):
                        nc.tensor.matmul(pb, R_sb[:, qi, :], S1t[:, qi, sl],
                                         start=(qi == 0), stop=(qi == NQT - 1))
                    cb = sb3.tile([128, 512], F32, name="cb")[:, 0:cw]
                    last = b == BL - 1 and cj == NCJ - 1
                    if last or b == 0:
                        nc.vector.tensor_tensor(cb, C_sb[:, sl].bitcast(F32),
                                                pb, ALU.mult)
                    else:
                        nc.scalar.activation(cb, pb, AF.Copy)
                        nc.gpsimd.tensor_tensor(cb, C_sb[:, sl].bitcast(F32),
                                                cb, ALU.mult)
                    nc.sync.dma_start(out=out[b, 384:512, sl], in_=cb)
                return go

            if part == "a":
                ops = []
                for cj in range(NCJ):
                    for qi in range(NQT):
                        ops.append(t_slice(cj, qi))
                    ops.append(pa_slice(cj))
                    if cj == 1:
                        ops.append(lambda: nc.sync.dma_start(
                            out=out[b, 0:128, :], in_=C_sb.bitcast(F32)))
                return ops
            ops = []
            for qi in range(NQT):
                ops.append(r_slice(qi, 0))
                ops.append(r_slice(qi, 1))
            for cj in range(NCJ):
                if b == BL - 1 and cj == NCJ - 1:
                    ops.append(pb_slice(cj, lo=1536, cw=256))
                    ops.append(pb_slice(cj, lo=1792, cw=256))
                else:
                    ops.append(pb_slice(cj))
            return ops

        # ---------------- emission schedule ----------------
        emit_loads(0)
        emit_prep(0)
        emit_loads(1)
        w0a = pass2_work(0, "a")
        w0a_sizes = [5, 6, 5, 5]

        def hook0(cj):
            n = w0a_sizes[cj]
            got, w0a[:n] = w0a[:n], []
            return got

        # b0's Ct/Qt quads fill b0's spine slack
        rem0 = emit_sloop(0, emit_ctqt(0))
        emit_prep(1)
        w0 = rem0 + w0a + pass2_work(0, "b") + emit_ctqt(1)
        w1a = pass2_work(1, "a")
        w1a_sizes = [5, 6, 5, 5]

        def hook1(cj):
            # b1's chunk-cj transposes and A-half become poppable once the
            # chunk's diags exist
            n = w1a_sizes[cj]
            got, w1a[:n] = w1a[:n], []
            return got

        emit_sloop(1, w0, chunk_hook=hook1)
        for op in w0:
            op()
        for op in w1a:
            op()
        for op in pass2_work(1, "b"):
            op()

    nc.finalize()
    return nc


_NC = None


def _get_nc():
    global _NC
    if _NC is None:
        _NC = _build_nc()
    return _NC


def kernel(C, Q, Cmask, Qmask, w4C, w4Q, w4mlu, bias, _trace=False):
    C = np.ascontiguousarray(np.asarray(C, dtype=np.float32))
    Q = np.ascontiguousarray(np.asarray(Q, dtype=np.float32))
    Cmask = np.ascontiguousarray(np.asarray(Cmask, dtype=np.int32))
    Qmask = np.ascontiguousarray(np.asarray(Qmask, dtype=np.int32))
    w4C = np.ascontiguousarray(np.asarray(w4C, dtype=np.float32))
    w4Q = np.ascontiguousarray(np.asarray(w4Q, dtype=np.float32))
    w4mlu = np.ascontiguousarray(np.asarray(w4mlu, dtype=np.float32))
    bias = np.ascontiguousarray(np.asarray(bias, dtype=np.float32))

    nc = _get_nc()
    in_maps = []
    for i in range(NCORES):
        s = slice(i * BL, (i + 1) * BL)
        in_maps.append({
            "C": C[s], "Q": Q[s], "Cmask": Cmask[s], "Qmask": Qmask[s],
            "w4C": w4C, "w4Q": w4Q, "w4mlu": w4mlu, "bias": bias,
        })
    res = run_bass_kernel_spmd(nc, in_maps, core_ids=list(range(NCORES)),
                               trace=_trace)
    out = np.concatenate([r["out"] for r in res.results], axis=0)
    if _trace:
        kernel._last_results = res
    return out


# revision 67
# speedup vs baseline: 1.1102x; 1.1102x over previous
"""CQAttention Trainium2 kernel (v4).

Reference per batch b (C:[D,Lc], Q:[D,Lq], D=128, Lc=2048, Lq=512):
    Ct = C^T, Qt = Q^T
    S  = Ct@w4C + (Qt@w4Q)^T + (Ct*w4mlu)@Qt^T + bias        [Lc, Lq]
    S1 = softmax_q(S + NEG*(1-qmask)),  S2 = softmax_c(S + NEG*(1-cmask))
    A  = S1 @ Qt;  B = S1 @ (S2^T @ Ct)
    out= transpose(concat([Ct, A, Ct*A, Ct*B], -1))           [4D, Lc]

Kernel algebra (single exp pass; one full-S exponential family):
    qterm[q] = q1[q] + qneg[q]   (rank-0/1 matmuls; PSUM-seeded per S tile)
    X[c,q]   = exp(sub2[c,q] + qterm[q] + c0[c] + cneg[c])
    rowsum[c]= sum_q X[c,q]              (free via ActE accum_out on the exp)
    S1t[q,c] = X^T[q,c] / rowsum[c]      (diag(rcp) matmul-transpose: a plain
                                          bf16 matmul, stationary X-block
                                          shared with the R matmuls)
    R        = (X wts vs [Ct|1]) -> rp/(s2+eps)
    A^T      = Qt^T @ S1t ;  B^T = R^T @ S1t
  `bias` cancels in both softmaxes (dropped exactly). The per-c factor
  exp(c0+cneg) cancels in S1's rowsum division; per-q factor exp(qterm)
  cancels in R's s2 division, so one exp family serves both softmaxes.
  qmask semantics are exact; for cmask=0 columns A/CA/CB deviate from the
  reference (0 instead of unmasked-softmax values); the input spec fixes
  Cmask=ones.

Scheduling: the program is emitted in two passes (loads/prep/S for all
batches, then R/transpose/AB for all batches) so that per-name tile-pool
slot rotation lets batch 1's front overlap batch 0's tail. CA/CB products
run on the otherwise-idle Pool engine from SBUF copies of A/B (the A copy
also feeds the output DMA).
"""

import numpy as np
from contextlib import ExitStack

import concourse.bass as bass
import concourse.mybir as mybir
import concourse.tile as tile
from concourse import bacc
from concourse.bass_utils import run_bass_kernel_spmd
from concourse.masks import make_identity

F32 = mybir.dt.float32
F32R = mybir.dt.float32r
I32 = mybir.dt.int32
BF16 = mybir.dt.bfloat16
AF = mybir.ActivationFunctionType
ALU = mybir.AluOpType

B, D, LC, LQ = 16, 128, 2048, 512
NCORES = 8
BL = B // NCORES          # batches per core
NEG = -1e30
NCT = LC // 128           # 16 c-tiles
NQT = LQ // 128           # 4 q-tiles
NCJ = LC // 512           # 4 c-chunks (free-dim)
CTS = 130                 # Ct slot: [Ct(128) | ones(1) | pad(1)]
EPS = 1e-30


def _build_nc():
    nc = bacc.Bacc("TRN2", target_bir_lowering=False)
    Ci = nc.dram_tensor("C", [BL, D, LC], F32, kind="ExternalInput")
    Qi = nc.dram_tensor("Q", [BL, D, LQ], F32, kind="ExternalInput")
    CMi = nc.dram_tensor("Cmask", [BL, LC], I32, kind="ExternalInput")
    QMi = nc.dram_tensor("Qmask", [BL, LQ], I32, kind="ExternalInput")
    w4C = nc.dram_tensor("w4C", [D, 1], F32, kind="ExternalInput")
    w4Q = nc.dram_tensor("w4Q", [D, 1], F32, kind="ExternalInput")
    w4mlu = nc.dram_tensor("w4mlu", [1, 1, D], F32, kind="ExternalInput")
    biasi = nc.dram_tensor("bias", [1], F32, kind="ExternalInput")
    out = nc.dram_tensor("out", [BL, 4 * D, LC], F32, kind="ExternalOutput")

    with tile.TileContext(nc) as tc, ExitStack() as ctx:
        const = ctx.enter_context(tc.tile_pool(name="const", bufs=1))
        sb2 = ctx.enter_context(tc.tile_pool(name="sb2", bufs=2))
        sb3 = ctx.enter_context(tc.tile_pool(name="sb3", bufs=3))
        # PSUM budget (8 banks): sp2 + pt2 + pab2 + sm1 + rp1
        ps_s = ctx.enter_context(tc.tile_pool(name="ps_s", bufs=2, space="PSUM"))
        ps_t = ctx.enter_context(tc.tile_pool(name="ps_t", bufs=2, space="PSUM"))
        ps_ab = ctx.enter_context(tc.tile_pool(name="ps_ab", bufs=2, space="PSUM"))
        ps_sm = ctx.enter_context(tc.tile_pool(name="ps_sm", bufs=1, space="PSUM"))
        ps_r = ctx.enter_context(tc.tile_pool(name="ps_r", bufs=1, space="PSUM"))

        # ---- constants (shared across batches) ----
        wmlu_sb = const.tile([D, 1], F32, name="wmlu_sb")
        nc.gpsimd.dma_start(out=wmlu_sb, in_=w4mlu.ap().rearrange("a b d -> d (a b)"))
        w4C_sb = const.tile([D, 1], F32R, name="w4C_sb")
        nc.gpsimd.dma_start(out=w4C_sb, in_=w4C[:, :].bitcast(F32R))
        w4Q_sb = const.tile([D, 1], F32R, name="w4Q_sb")
        nc.scalar.dma_start(out=w4Q_sb, in_=w4Q[:, :].bitcast(F32R))
        ident0 = const.tile([D, D], F32, name="ident0")
        make_identity(nc, ident0)
        identR = const.tile([D, D], F32R, name="identR")
        nc.vector.tensor_copy(identR, ident0)
        identB = const.tile([D, D], BF16, name="identB")
        nc.vector.tensor_copy(identB, ident0)
        id16 = const.tile([16, 16], F32, name="id16")
        make_identity(nc, id16)
        ones_f = const.tile([1, D], F32, name="ones_f")
        nc.vector.memset(ones_f, 1.0)
        one1R = const.tile([1, 1], F32R, name="one1R")
        nc.vector.tensor_copy(one1R, ones_f[:, 0:1])
        ones_row = const.tile([1, D], F32R, name="ones_row")
        nc.vector.tensor_copy(ones_row, ones_f)
        ones_col = const.tile([D, 1], F32, name="ones_col")
        nc.vector.memset(ones_col, 1.0)

        st = [dict() for _ in range(BL)]

        def emit_loads(b):
            s = st[b]
            # All loads ride the sync queue (SP has no compute to block),
            # masks first: they head the bias/qterm dependency chains.
            C_sb = s["C_sb"] = sb2.tile([D, LC], F32R, name="C_sb")
            Q_sb = s["Q_sb"] = sb2.tile([D, LQ], F32R, name="Q_sb")
            nc.sync.dma_start(out=Q_sb, in_=Qi[b, :, :].bitcast(F32R))
            if b == 0:
                nc.sync.dma_start(out=C_sb[:, 0:512],
                                  in_=Ci[b, :, 0:512].bitcast(F32R))
            s["qm_row"] = sb2.tile([1, LQ], I32, name="qm_row")
            nc.sync.dma_start(out=s["qm_row"], in_=QMi[b, :].unsqueeze(0))
            s["cm_pi"] = sb2.tile([16, 128], I32, name="cm_pi")
            nc.gpsimd.dma_start(out=s["cm_pi"],
                                in_=CMi[b, :].rearrange("(p i) -> p i", p=16))
            if b == 0:
                nc.sync.dma_start(out=C_sb[:, 512:2048],
                                  in_=Ci[b, :, 512:2048].bitcast(F32R))
            else:
                nc.sync.dma_start(out=C_sb, in_=Ci[b, :, :].bitcast(F32R))

        def emit_prep(b):
            # Q/mask-dependent prep (ready before C lands)
            s = st[b]
            Q_sb = s["Q_sb"]
            qneg_row = sb2.tile([1, LQ], F32R, name="qneg_row")
            nc.vector.tensor_scalar(qneg_row, s["qm_row"], -NEG, NEG,
                                    op0=ALU.mult, op1=ALU.add)
            cn_t = sb2.tile([16, 128], F32, name="cn_t")
            nc.vector.tensor_scalar(cn_t, s["cm_pi"], -NEG, NEG,
                                    op0=ALU.mult, op1=ALU.add)
            cnT_sb = s["cnT_sb"] = sb2.tile([128, 16], F32, name="cnT_sb")
            cnT_p = ps_sm.tile([128, 16], F32, name="sm")
            nc.tensor.transpose(cnT_p, cn_t, id16)
            nc.vector.tensor_copy(cnT_sb, cnT_p)
            qt_p = ps_sm.tile([1, LQ], F32, name="sm")
            nc.tensor.matmul(qt_p, w4Q_sb, Q_sb, start=True, stop=False)
            nc.tensor.matmul(qt_p, one1R, qneg_row, start=False, stop=True)
            qterm_row = s["qterm_row"] = sb2.tile([1, LQ], F32R,
                                                  name="qterm_row")
            nc.scalar.activation(qterm_row, qt_p, AF.Copy)

        def emit_sloop(b, work, chunk_hook=None):
            # S phase, chunk-interleaved with C's arrival; after each S tile
            # pop one deferred-work closure to fill PE's exp-cadence slack.
            s = st[b]
            C_sb, Q_sb, qterm_row = s["C_sb"], s["Q_sb"], s["qterm_row"]
            cnT_sb = s["cnT_sb"]
            bias_c = sb2.tile([128, NCT], F32, name="bias_c")
            Cw = sb2.tile([D, LC], F32R, name="Cw")
            X = s["X"] = sb2.tile([128, NCT, LQ], BF16, name="X")
            rowsum = sb2.tile([128, NCT], F32, name="rowsum")
            rcp = sb2.tile([128, NCT], F32, name="rcp")
            diags = s["diags"] = sb2.tile([128, NCT, 128], BF16, name="diags")
            for cj in range(NCJ):
                sl = slice(cj * 512, (cj + 1) * 512)
                qs = slice(cj * 4, (cj + 1) * 4)
                c0_p = ps_sm.tile([128, 4], F32, name="sm")
                for k in range(4):
                    ci = cj * 4 + k
                    nc.tensor.matmul(
                        c0_p[:, k : k + 1],
                        C_sb.bitcast(F32)[:, ci * 128 : (ci + 1) * 128],
                        w4C_sb.bitcast(F32), start=True, stop=True)
                nc.vector.tensor_tensor(bias_c[:, qs], c0_p, cnT_sb[:, qs],
                                        ALU.add)
                nc.vector.tensor_scalar_mul(Cw[:, sl],
                                            C_sb[:, sl].bitcast(F32),
                                            wmlu_sb[:, 0:1])
                for k in range(4):
                    ci = cj * 4 + k
                    sp = ps_s.tile([128, LQ], F32, name="sp")
                    nc.tensor.matmul(sp, ones_row, qterm_row,
                                     start=True, stop=False)
                    nc.tensor.matmul(sp, Cw[:, ci * 128 : (ci + 1) * 128],
                                     Q_sb, start=False, stop=True)
                    nc.scalar.activation(X[:, ci, :], sp, AF.Exp,
                                         bias=bias_c[:, ci : ci + 1],
                                         scale=1.0,
                                         accum_out=rowsum[:, ci : ci + 1])
                    if work:
                        work.pop(0)()
                    if work:
                        work.pop(0)()
                # eps guard: all-masked rows divide 0/eps -> 0, not NaN
                nc.vector.tensor_scalar_add(rowsum[:, qs], rowsum[:, qs], EPS)
                nc.vector.reciprocal(rcp[:, qs], rowsum[:, qs])
                for ck in range(cj * 4, cj * 4 + 4):
                    nc.vector.tensor_scalar_mul(diags[:, ck, :], identB,
                                                rcp[:, ck : ck + 1])
                if chunk_hook is not None:
                    work.extend(chunk_hook(cj))
            return work

        def emit_ctqt(b):
            # Ct/Qt tiles (bf16, batched PSUM->SBUF copies)
            s = st[b]
            C_sb, Q_sb = s["C_sb"], s["Q_sb"]
            Ct_sb = s["Ct_sb"] = sb2.tile([128, NCT, CTS], BF16, name="Ct_sb")
            nc.gpsimd.tensor_copy(
                Ct_sb[:, :, 128:129],
                ones_col[:, 0:1].unsqueeze(1).to_broadcast((128, NCT, 1)))

            def quad(cq):
                def go():
                    tp = ps_t.tile([128, 4, 128], F32R, name="pt")
                    for k in range(4):
                        ci = cq * 4 + k
                        nc.tensor.transpose(tp[:, k, :],
                                            C_sb[:, ci * 128 : (ci + 1) * 128],
                                            identR)
                    nc.vector.tensor_copy(
                        Ct_sb[:, cq * 4 : (cq + 1) * 4, 0:128],
                        tp.bitcast(F32))
                return go

            def qt():
                def go():
                    Qt_sb = s["Qt_sb"] = sb2.tile([128, NQT, 128], BF16,
                                                  name="Qt_sb")
                    tpq = ps_t.tile([128, NQT, 128], F32R, name="pt")
                    for qi in range(NQT):
                        nc.tensor.transpose(
                            tpq[:, qi, :],
                            Q_sb[:, qi * 128 : (qi + 1) * 128], identR)
                    nc.vector.tensor_copy(Qt_sb, tpq.bitcast(F32))
                return go
            return [quad(cq) for cq in range(4)] + [qt()]

        def pass2_work(b, part):
            # Deferred R/S1t-transpose/A/B slices (~0.5-1us of PE each)
            s = st[b]
            C_sb = s["C_sb"]
            if "R_sb" not in s:
                s["R_sb"] = sb2.tile([128, NQT, 128], BF16, name="R_sb")
                s["rs2"] = sb2.tile([128, NQT], F32, name="rs2")
                s["S1t"] = sb2.tile([128, NQT, LC], BF16, name="S1t")
            R_sb, rs2, S1t = s["R_sb"], s["rs2"], s["S1t"]

            rphold = s.setdefault("rphold", {})

            def r_slice(qi, half):
                def go():
                    if half == 0:
                        rphold[qi] = ps_r.tile([128, CTS], F32, name="rp")
                    rp = rphold[qi]
                    for ci in range(half * 8, half * 8 + 8):
                        nc.tensor.matmul(rp,
                                         s["X"][:, ci, qi * 128 : (qi + 1) * 128],
                                         s["Ct_sb"][:, ci, 0:CTS],
                                         start=(ci == 0), stop=(ci == NCT - 1))
                    if half == 1:
                        nc.vector.tensor_scalar_add(rs2[:, qi : qi + 1],
                                                    rp[:, 128:129], EPS)
                        nc.vector.reciprocal(rs2[:, qi : qi + 1],
                                             rs2[:, qi : qi + 1])
                        nc.vector.tensor_scalar_mul(R_sb[:, qi, :],
                                                    rp[:, 0:128],
                                                    rs2[:, qi : qi + 1])
                return go

            def t_slice(cj, qi):
                def go():
                    if b == 1 and (cj + qi) % 2 == 1:
                        tp = ps_s.tile([128, 512], F32, name="sp", tag="sp")
                    else:
                        tp = ps_t.tile([128, 512], F32, name="pt")
                    for k in range(4):
                        ci = cj * 4 + k
                        nc.tensor.matmul(
                            tp[:, k * 128 : (k + 1) * 128],
                            s["X"][:, ci, qi * 128 : (qi + 1) * 128],
                            s["diags"][:, ci, :], start=True, stop=True)
                    on_dve = ((qi * NCJ + cj) % 4 != 3 if b == 0
                              else (qi + cj) % 2 == 0)
                    if on_dve:
                        nc.vector.tensor_copy(
                            S1t[:, qi, cj * 512 : (cj + 1) * 512], tp)
                    else:
                        nc.scalar.activation(
                            S1t[:, qi, cj * 512 : (cj + 1) * 512], tp,
                            AF.Copy)
                return go

            def pa_slice(cj):
                def go():
                    sl = slice(cj * 512, (cj + 1) * 512)
                    pa = ps_ab.tile([128, 512], F32, name="pab")
                    for qi in range(NQT):
                        nc.tensor.matmul(pa, s["Qt_sb"][:, qi, :],
                                         S1t[:, qi, sl],
                                         start=(qi == 0), stop=(qi == NQT - 1))
                    cab = sb3.tile([128, 2, 512], F32, name="cab")
                    if b == 0:
                        nc.vector.tensor_copy(cab[:, 0, :], pa)
                        nc.gpsimd.tensor_tensor(cab[:, 1, :],
                                                C_sb[:, sl].bitcast(F32),
                                                cab[:, 0, :], ALU.mult)
                    else:
                        nc.scalar.activation(cab[:, 0, :], pa, AF.Copy)
                        nc.gpsimd.tensor_tensor(cab[:, 1, :],
                                                C_sb[:, sl].bitcast(F32),
                                                cab[:, 0, :], ALU.mult)
                    nc.sync.dma_start(
                        out=out[b, 128:384, sl].rearrange("(r p) c -> p r c",
                                                          p=128),
                        in_=cab)
                return go

            def pb_slice(cj, lo=None, cw=512):
                def go():
                    sl = (slice(cj * 512, (cj + 1) * 512) if lo is None
                          else slice(lo, lo + cw))
                    pb = ps_ab.tile([128, 512], F32, name="pab")[:, 0:cw]
                    for qi in range(NQT):
                        nc.tensor.matmul(pb, R_sb[:, qi, :], S1t[:, qi, sl],
                                         start=(qi == 0), stop=(qi == NQT - 1))
                    cb = sb3.tile([128, 512], F32, name="cb")[:, 0:cw]
                    last = b == BL - 1 and cj == NCJ - 1
                    if last or b == 0:
                        nc.vector.tensor_tensor(cb, C_sb[:, sl].bitcast(F32),
                                                pb, ALU.mult)
                    else:
                        nc.scalar.activation(cb, pb, AF.Copy)
                        nc.gpsimd.tensor_tensor(cb, C_sb[:, sl].bitcast(F32),
                                                cb, ALU.mult)
                    nc.sync.dma_start(out=out[b, 384:512, sl], in_=cb)
                return go

            if part == "a":
                ops = []
                for cj in range(NCJ):
                    for qi in range(NQT):
                        ops.append(t_slice(cj, qi))
                    ops.append(pa_slice(cj))
                    if cj == 1:
                        ops.append(lambda: nc.sync.dma_start(
                            out=out[b, 0:128, :], in_=C_sb.bitcast(F32)))
                return ops
            ops = []
            for qi in range(NQT):
                ops.append(r_slice(qi, 0))
                ops.append(r_slice(qi, 1))
            for cj in range(NCJ):
                if b == BL - 1 and cj == NCJ - 1:
                    ops.append(pb_slice(cj, lo=1536, cw=256))
                    ops.append(pb_slice(cj, lo=1792, cw=256))
                else:
                    ops.append(pb_slice(cj))
            return ops

        # ---------------- emission schedule ----------------
        emit_loads(0)
        emit_prep(0)
        emit_loads(1)
        w0a = pass2_work(0, "a")
        w0a_sizes = [5, 6, 5, 5]

        def hook0(cj):
            n = w0a_sizes[cj]
            got, w0a[:n] = w0a[:n], []
            return got

        # b0's Ct/Qt quads fill b0's spine slack
        rem0 = emit_sloop(0, emit_ctqt(0))
        emit_prep(1)
        w0 = rem0 + w0a + pass2_work(0, "b") + emit_ctqt(1)
        w1a = pass2_work(1, "a")
        w1a_sizes = [5, 6, 5, 5]

        def hook1(cj):
            # b1's chunk-cj transposes and A-half become poppable once the
            # chunk's diags exist
            n = w1a_sizes[cj]
            got, w1a[:n] = w1a[:n], []
            return got

        emit_sloop(1, w0, chunk_hook=hook1)
        for op in w0:
            op()
        for op in w1a:
            op()
        for op in pass2_work(1, "b"):
            op()

    nc.finalize()
    return nc


_NC = None


def _get_nc():
    global _NC
    if _NC is None:
        _NC = _build_nc()
    return _NC


def kernel(C, Q, Cmask, Qmask, w4C, w4Q, w4mlu, bias, _trace=False):
    C = np.ascontiguousarray(np.asarray(C, dtype=np.float32))
    Q = np.ascontiguousarray(np.asarray(Q, dtype=np.float32))
    Cmask = np.ascontiguousarray(np.asarray(Cmask, dtype=np.int32))
    Qmask = np.ascontiguousarray(np.asarray(Qmask, dtype=np.int32))
    w4C = np.ascontiguousarray(np.asarray(w4C, dtype=np.float32))
    w4Q = np.ascontiguousarray(np.asarray(w4Q, dtype=np.float32))
    w4mlu = np.ascontiguousarray(np.asarray(w4mlu, dtype=np.float32))
    bias = np.ascontiguousarray(np.asarray(bias, dtype=np.float32))

    nc = _get_nc()
    in_maps = []
    for i in range(NCORES):
        s = slice(i * BL, (i + 1) * BL)
        in_maps.append({
            "C": C[s], "Q": Q[s], "Cmask": Cmask[s], "Qmask": Qmask[s],
            "w4C": w4C, "w4Q": w4Q, "w4mlu": w4mlu, "bias": bias,
        })
    res = run_bass_kernel_spmd(nc, in_maps, core_ids=list(range(NCORES)),
                               trace=_trace)
    out = np.concatenate([r["out"] for r in res.results], axis=0)
    if _trace:
        kernel._last_results = res
    return out
